# revision 1
# baseline (speedup 1.0000x reference)
"""Trainium2 Bass kernel for nn_Attn (Bahdanau-style attention scores).

Reference computation:
    energy[s,b,:] = W @ enc[s,b,:] + bias          [S,B,H]
    scores[b,s]   = hidden[0,b,:] . energy[s,b,:]  [B,S]
    out           = softmax(scores, axis=-1)[:,None,:]

Key rewrite: scores[b,s] = (W^T hidden_b) . enc[s,b,:] + hidden_b . bias.
The second term is constant in s, so it is invariant under softmax and is
dropped entirely.  v_b = W^T hidden_b is a tiny [B, 2H] matvec done on the
tensor engine; the S*B*2H dot-product sweep is done by the vector engine
(elementwise multiply) + scalar engine (activation-Copy with accum_out for
the free-dim sum) while DMA streams enc at full HBM bandwidth.

Sharding: data-parallel over batch B (4 batch rows per core, 8 cores).
Each core receives enc[:, b0:b0+4, :] (64 MiB), hidden^T slice and W.
"""

import numpy as np

# Problem sizes (hardcoded per harness contract).
H = 1024          # hidden size
K = 2 * H         # 2H = contraction dim of W
S = 2048          # encoder sequence length
B = 32            # batch
N_CORES = 8
BPC = B // N_CORES  # batch rows per core = 4

ST = 128          # s-tile (partition dim)
NST = S // ST     # 16 s-tiles
KC = 512          # psum free chunk for the v matmul
NKC = K // KC     # 4
HC = 128          # h chunk (matmul contraction tile)
NHC = H // HC     # 8
BGRP = 2          # batch rows per enc DMA tile

# debug toggles (bisect)
USE_GPSIMD_RING = False  # enc DMAs also on SWDGE ring (slower: Q7 chokes)
USE_NEG_REDUCE = True    # tensor_reduce(negate=True)
USE_PE_TAIL = True       # transposed-softmax tail (vs per-partition path)

_CACHE = {}


def _emit(ctx, tc, enc, hidT, w, out):
    """Emit the per-core program.

    enc : DRAM [S, BPC, K]  fp32
    hidT: DRAM [128, NHC*BPC] fp32, layout [p][c][b] for h = c*128 + p
    w   : DRAM [H, K] fp32
    out : DRAM [BPC, S] fp32  (softmax probabilities)
    """
    from concourse import mybir
    from concourse.masks import make_identity

    nc = tc.nc
    f32 = mybir.dt.float32

    singles = ctx.enter_context(tc.tile_pool(name="singles", bufs=1))
    wpool = ctx.enter_context(tc.tile_pool(name="wpool", bufs=2))
    encpool = ctx.enter_context(tc.tile_pool(name="encp", bufs=4))
    prodpool = ctx.enter_context(tc.tile_pool(name="prodp", bufs=3))
    vpsum = ctx.enter_context(tc.tile_pool(name="vpsum", bufs=1, space="PSUM"))
    bcpsum = ctx.enter_context(tc.tile_pool(name="bcpsum", bufs=2, space="PSUM"))
    tpsum = ctx.enter_context(tc.tile_pool(name="tpsum", bufs=1, space="PSUM"))
    small = ctx.enter_context(tc.tile_pool(name="small", bufs=2))

    # ---- constants (no input deps; scheduled early) ---------------------
    ident = singles.tile([128, 128], f32)
    make_identity(nc, ident)
    ones = singles.tile([1, 128], f32)
    nc.vector.memset(ones, 1.0)

    # ---- PE warm-up ------------------------------------------------------
    # TensorE clocks at 1.2 GHz until it has been busy ~4us, then 2.4 GHz.
    # The v chain is PE-bound, so burn dummy matmuls on a scratch PSUM bank
    # while the W DMAs stream: the real matmuls then run at full clock.
    warm_ps = bcpsum.tile([128, 128], f32, name="warm_ps", tag="warm_ps")
    for _ in range(36):
        nc.tensor.matmul(
            warm_ps[:, :], lhsT=ident, rhs=ident, start=True, stop=True
        )

    # ---- load hidden^T (tiny) -------------------------------------------
    hid_sb = singles.tile([128, NHC * BPC], f32)
    nc.scalar.dma_start(out=hid_sb, in_=hidT)

    # ---- v = W^T h, quarter-by-quarter over k, fused with broadcast -----
    # W streams as 4 column-quarter tiles [128, NHC, KC]; quarter q's
    # matvec + partition-0 flatten + ones-matmul broadcast overlap the DMA
    # of quarter q+1, so v_bc completes right after the last W byte lands.
    v_bc = singles.tile([128, BPC, K], f32)
    v_sb = singles.tile([BPC, K], f32)
    w_dmas = []
    for q in range(NKC):
        w_sb = wpool.tile([128, NHC, KC], f32, name="w_sb", tag="w_sb")
        weng = nc.scalar if (q % 2 == 0) else nc.sync
        w_dmas.append(
            weng.dma_start(
                out=w_sb,
                in_=w[:, q * KC:(q + 1) * KC].rearrange("(c p) k -> p c k", p=HC),
            )
        )
        v_ps = vpsum.tile([BPC, KC], f32, name="v_ps", tag="v_ps", bufs=2)
        for c in range(NHC):
            nc.tensor.matmul(
                v_ps[:, :],
                lhsT=hid_sb[:, c * BPC:(c + 1) * BPC],
                rhs=w_sb[:, c, :],
                start=(c == 0),
                stop=(c == NHC - 1),
            )
        nc.scalar.copy(out=v_sb[:, q * KC:(q + 1) * KC], in_=v_ps[:, :])
        # flatten the 4 v rows of this quarter onto partition 0
        v_row = singles.tile([1, BPC * KC], f32, name="v_row", tag="v_row")
        nc.gpsimd.dma_start(out=v_row, in_=v_sb[:, q * KC:(q + 1) * KC])
        for b in range(BPC):
            bc_ps = bcpsum.tile([128, KC], f32, name="bc_ps", tag="bc_ps")
            nc.tensor.matmul(
                bc_ps[:, :],
                lhsT=ones,
                rhs=v_row[0:1, b * KC:(b + 1) * KC],
                start=True,
                stop=True,
            )
            eng = nc.vector if (q * BPC + b) % 2 == 0 else nc.scalar
            if eng is nc.vector:
                eng.tensor_copy(v_bc[:, b, q * KC:(q + 1) * KC], bc_ps[:, :])
            else:
                eng.copy(out=v_bc[:, b, q * KC:(q + 1) * KC], in_=bc_ps[:, :])

    # ---- main sweep: scores[s,b] = enc[s,b,:] . v_b ---------------------
    # DVE does the elementwise multiply; ScalarE (activation Copy with
    # accum_out) does the free-dim sum, so the two passes run on separate
    # engines and both stay under the DMA streaming time.
    scores = singles.tile([128, BPC, NST], f32)
    NBG = BPC // BGRP
    # All bulk enc DMAs issue from the sync engine: its sequencer does
    # nothing else, so descriptor generation is never delayed behind
    # compute (scalar's sequencer is saturated by the reduce chain).
    enc_rings = [nc.sync]
    from concourse.bass import _add_dep_helper

    for st in range(NST):
        for g in range(NBG):
            enc_sb = encpool.tile([128, BGRP, K], f32)
            eng = enc_rings[(st * NBG + g) % len(enc_rings)]
            enc_dma = eng.dma_start(
                out=enc_sb,
                in_=enc[st * ST:(st + 1) * ST, g * BGRP:(g + 1) * BGRP, :],
            )
            i = st * NBG + g
            if i < 4:
                # the W phase is DMA-bound (~19us at full rate): hold the
                # first enc DMAs until every W quarter has landed so enc
                # traffic never delays the v chain on the rings
                _add_dep_helper(
                    enc_dma.ins,
                    w_dmas[-1].ins,
                    reason="enc stream yields to W prologue",
                )
            for bi in range(BGRP):
                b = g * BGRP + bi
                prod = prodpool.tile([128, K], f32, name="prod", tag="prod")
                nc.vector.tensor_mul(prod, enc_sb[:, bi, :], v_bc[:, b, :])
                nc.scalar.activation(
                    out=prod,
                    in_=prod,
                    func=mybir.ActivationFunctionType.Copy,
                    bias=0.0,
                    scale=1.0,
                    accum_out=scores[:, b, st:st + 1],
                )

    # ---- softmax over s, in transposed [BPC, S] layout ------------------
    # scores [128 s_in, (b t)] -> PE transpose -> [(b t), s_in] -> SBUF->SBUF
    # DMA reshape -> s4 [BPC, S].  Then softmax is a single free-axis chain:
    # -max (negated reduce), in-place exp with bias + fused denominator
    # accum, reciprocal, in-place scale, natural-layout store.
    sc2 = scores.rearrange("p b t -> p (b t)")
    scT_ps = tpsum.tile([BPC * NST, 128], f32)
    nc.tensor.transpose(scT_ps[:, :], sc2, ident[:, :])
    scT = small.tile([BPC * NST, 128], f32)
    nc.vector.tensor_copy(scT, scT_ps[:, :])
    s4 = singles.tile([BPC, S], f32)
    nc.sync.dma_start(out=s4, in_=scT)

    nm4 = small.tile([BPC, 1], f32)
    if USE_NEG_REDUCE:
        nc.vector.tensor_reduce(
            out=nm4, in_=s4, axis=mybir.AxisListType.X, op=mybir.AluOpType.max,
            negate=True,
        )
    else:
        m4 = small.tile([BPC, 1], f32)
        nc.vector.tensor_reduce(
            out=m4, in_=s4, axis=mybir.AxisListType.X, op=mybir.AluOpType.max
        )
        nc.vector.tensor_scalar_mul(nm4, m4, -1.0)
    r4 = small.tile([BPC, 1], f32)
    nc.scalar.activation(
        out=s4,
        in_=s4,
        func=mybir.ActivationFunctionType.Exp,
        bias=nm4,
        scale=1.0,
        accum_out=r4,
    )
    inv4 = small.tile([BPC, 1], f32)
    nc.vector.reciprocal(inv4, r4)
    nc.vector.tensor_scalar_mul(s4, s4, inv4)
    nc.sync.dma_start(out=out, in_=s4)


def _build():
    if "nc" in _CACHE:
        return _CACHE["nc"]
    from contextlib import ExitStack

    import concourse.bacc as bacc
    import concourse.tile as tile
    from concourse import mybir

    nc = bacc.Bacc(
        "TRN2", target_bir_lowering=False, debug=False, num_devices=N_CORES
    )
    enc_d = nc.dram_tensor("enc", [S, BPC, K], mybir.dt.float32, kind="ExternalInput")
    hid_d = nc.dram_tensor(
        "hidT", [128, NHC * BPC], mybir.dt.float32, kind="ExternalInput"
    )
    w_d = nc.dram_tensor("w", [H, K], mybir.dt.float32, kind="ExternalInput")
    out_d = nc.dram_tensor(
        "attn_out", [BPC, S], mybir.dt.float32, kind="ExternalOutput"
    )

    with tile.TileContext(nc) as tc:
        with ExitStack() as ctx:
            _emit(ctx, tc, enc_d.ap(), hid_d.ap(), w_d.ap(), out_d.ap())
    nc.compile()
    _CACHE["nc"] = nc
    return nc


def _make_in_maps(hidden, encoder_outputs, W):
    in_maps = []
    w = np.ascontiguousarray(W, dtype=np.float32)
    for i in range(N_CORES):
        b0 = i * BPC
        # hidT layout [p][c][b] with h = c*128 + p
        hid = hidden[0, b0:b0 + BPC, :]                    # [BPC, H]
        hidT = np.ascontiguousarray(
            hid.T.reshape(NHC, 128, BPC).transpose(1, 0, 2).reshape(128, NHC * BPC),
            dtype=np.float32,
        )
        enc = np.ascontiguousarray(
            encoder_outputs[:, b0:b0 + BPC, :], dtype=np.float32
        )
        in_maps.append({"enc": enc, "hidT": hidT, "w": w})
    return in_maps


def kernel(hidden, encoder_outputs, W, b):
    from concourse import bass_utils

    nc = _build()
    in_maps = _make_in_maps(
        np.asarray(hidden), np.asarray(encoder_outputs), np.asarray(W)
    )
    res = bass_utils.run_bass_kernel_spmd(
        nc, in_maps, core_ids=list(range(N_CORES))
    )
    out = np.concatenate(
        [res.results[i]["attn_out"] for i in range(N_CORES)], axis=0
    )  # [B, S]
    return out[:, None, :].astype(np.float32)



# revision 2
# speedup vs baseline: 1.1740x; 1.1740x over previous
"""Trainium2 Bass kernel for nn_Attn (Bahdanau-style attention scores).

Reference computation:
    energy[s,b,:] = W @ enc[s,b,:] + bias          [S,B,H]
    scores[b,s]   = hidden[0,b,:] . energy[s,b,:]  [B,S]
    out           = softmax(scores, axis=-1)[:,None,:]

Key rewrite: scores[b,s] = (W^T hidden_b) . enc[s,b,:] + hidden_b . bias.
The second term is constant in s, so it is invariant under softmax and is
dropped entirely.  v_b = W^T hidden_b is a tiny [B, 2H] matvec done on the
tensor engine.

fp16 edition: enc, W, hidden and v are all fp16 (host-side cast), which
halves HBM traffic to ~36 MiB/core (the hard floor at ~330 GB/s is then
~110 us).  The S*B*2H dot-product sweep is split three ways so every
engine stays under the DMA streaming time:
  x-units: DVE fused scalar_tensor_tensor (mul + free-axis accum, 1x rate)
  y-units: DVE tensor_mul (fp16 2x rate) + ScalarE activation-Copy accum
  z-units: GpSimd tensor_mul + ScalarE activation-Copy accum
Measured-model budget: DVE ~95us, Scalar ~90us, GpSimd ~50us < DMA ~112us.

Sharding: data-parallel over batch B (4 batch rows per core, 8 cores).
Each core receives enc[:, b0:b0+4, :] as fp16 (32 MiB), hidden^T slice
and W as fp16.  Softmax tail unchanged (fp32).
"""

import numpy as np

# Problem sizes (hardcoded per harness contract).
H = 1024          # hidden size
K = 2 * H         # 2H = contraction dim of W
S = 2048          # encoder sequence length
B = 32            # batch
N_CORES = 8
BPC = B // N_CORES  # batch rows per core = 4

ST = 128          # s-tile (partition dim)
NST = S // ST     # 16 s-tiles
KC = 512          # psum free chunk for the v matmul
NKC = K // KC     # 4
HC = 128          # h chunk (matmul contraction tile)
NHC = H // HC     # 8
BGRP = 2          # batch rows per enc DMA tile

# Sweep engine split: per 16 stream units, which engine path handles each.
#   'x' = DVE fused mul+accum   'y' = DVE mul + Scalar accum
#   'z' = GpSimd mul + Scalar accum
MODE_PATTERN = ['x', 'y', 'z', 'x', 'y', 'x', 'y', 'x',
                'y', 'z', 'x', 'y', 'x', 'z', 'x', 'y']

_CACHE = {}


def _emit(ctx, tc, enc, hidT, w, out):
    """Emit the per-core program.

    enc : DRAM [S, BPC, K]  fp16
    hidT: DRAM [128, NHC*BPC] fp16, layout [p][c][b] for h = c*128 + p
    w   : DRAM [H, K] fp16
    out : DRAM [BPC, S] fp32  (softmax probabilities)
    """
    from concourse import mybir
    from concourse.masks import make_identity

    nc = tc.nc
    f32 = mybir.dt.float32
    f16 = mybir.dt.float16

    singles = ctx.enter_context(tc.tile_pool(name="singles", bufs=1))
    wpool = ctx.enter_context(tc.tile_pool(name="wpool", bufs=2))
    encpool = ctx.enter_context(tc.tile_pool(name="encp", bufs=8))
    prodpool = ctx.enter_context(tc.tile_pool(name="prodp", bufs=4))
    vpsum = ctx.enter_context(tc.tile_pool(name="vpsum", bufs=1, space="PSUM"))
    bcpsum = ctx.enter_context(tc.tile_pool(name="bcpsum", bufs=2, space="PSUM"))
    tpsum = ctx.enter_context(tc.tile_pool(name="tpsum", bufs=1, space="PSUM"))
    small = ctx.enter_context(tc.tile_pool(name="small", bufs=2))

    # ---- constants (no input deps; scheduled early) ---------------------
    ident = singles.tile([128, 128], f32)
    make_identity(nc, ident)
    ones16 = singles.tile([1, 128], f16)
    nc.vector.memset(ones16, 1.0)

    # ---- PE warm-up ------------------------------------------------------
    # TensorE clocks at 1.2 GHz until it has been busy ~3us, then 2.4 GHz.
    # Burn dummy matmuls on a scratch PSUM bank while the W DMAs stream.
    warm_ps = bcpsum.tile([128, 128], f32, name="warm_ps", tag="warm_ps")
    for _ in range(24):
        nc.tensor.matmul(
            warm_ps[:, :], lhsT=ident, rhs=ident, start=True, stop=True
        )

    # ---- load hidden^T (tiny, fp16) -------------------------------------
    hid_sb = singles.tile([128, NHC * BPC], f16)
    nc.scalar.dma_start(out=hid_sb, in_=hidT)

    # ---- v = W^T h, quarter-by-quarter over k, fused with broadcast -----
    # W streams as 4 column-quarter tiles [128, NHC, KC] fp16; quarter q's
    # matvec + partition-0 flatten + ones-matmul broadcast overlap the DMA
    # of quarter q+1, so v_bc completes right after the last W byte lands.
    v_bc = singles.tile([128, BPC, K], f16)
    v16_sb = singles.tile([BPC, K], f16)
    w_dmas = []
    for q in range(NKC):
        w_sb = wpool.tile([128, NHC, KC], f16, name="w_sb", tag="w_sb")
        weng = nc.scalar if (q % 2 == 0) else nc.sync
        w_dmas.append(
            weng.dma_start(
                out=w_sb,
                in_=w[:, q * KC:(q + 1) * KC].rearrange("(c p) k -> p c k", p=HC),
            )
        )
        v_ps = vpsum.tile([BPC, KC], f32, name="v_ps", tag="v_ps", bufs=2)
        for c in range(NHC):
            nc.tensor.matmul(
                v_ps[:, :],
                lhsT=hid_sb[:, c * BPC:(c + 1) * BPC],
                rhs=w_sb[:, c, :],
                start=(c == 0),
                stop=(c == NHC - 1),
            )
        # downcast to fp16 on the psum->sbuf copy
        nc.scalar.copy(out=v16_sb[:, q * KC:(q + 1) * KC], in_=v_ps[:, :])
        # flatten the 4 v rows of this quarter onto partition 0
        v_row = singles.tile([1, BPC * KC], f16, name="v_row", tag="v_row")
        nc.gpsimd.dma_start(out=v_row, in_=v16_sb[:, q * KC:(q + 1) * KC])
        for b in range(BPC):
            bc_ps = bcpsum.tile([128, KC], f32, name="bc_ps", tag="bc_ps")
            nc.tensor.matmul(
                bc_ps[:, :],
                lhsT=ones16,
                rhs=v_row[0:1, b * KC:(b + 1) * KC],
                start=True,
                stop=True,
            )
            eng = nc.vector if (q * BPC + b) % 2 == 0 else nc.scalar
            if eng is nc.vector:
                eng.tensor_copy(v_bc[:, b, q * KC:(q + 1) * KC], bc_ps[:, :])
            else:
                eng.copy(out=v_bc[:, b, q * KC:(q + 1) * KC], in_=bc_ps[:, :])

    # ---- main sweep: scores[s,b] = enc[s,b,:] . v_b ---------------------
    # 64 (s-tile, b) units split across DVE/Scalar/GpSimd per MODE_PATTERN.
    scores = singles.tile([128, BPC, NST], f32)
    NBG = BPC // BGRP
    from concourse.bass import _add_dep_helper

    unit = 0
    for st in range(NST):
        for g in range(NBG):
            enc_sb = encpool.tile([128, BGRP, K], f16)
            enc_dma = nc.sync.dma_start(
                out=enc_sb,
                in_=enc[st * ST:(st + 1) * ST, g * BGRP:(g + 1) * BGRP, :],
            )
            i = st * NBG + g
            if i < 4:
                # the W phase is DMA-bound (~13us at full rate): hold the
                # first enc DMAs until every W quarter has landed so enc
                # traffic never delays the v chain
                _add_dep_helper(
                    enc_dma.ins,
                    w_dmas[-1].ins,
                    reason="enc stream yields to W prologue",
                )
            for bi in range(BGRP):
                b = g * BGRP + bi
                mode = MODE_PATTERN[unit % len(MODE_PATTERN)]
                unit += 1
                prod = prodpool.tile([128, K], f16, name="prod", tag="prod")
                if mode == 'x':
                    nc.vector.scalar_tensor_tensor(
                        out=prod,
                        in0=enc_sb[:, bi, :],
                        scalar=1.0,
                        in1=v_bc[:, b, :],
                        op0=mybir.AluOpType.mult,
                        op1=mybir.AluOpType.mult,
                        accum_out=scores[:, b, st:st + 1],
                    )
                else:
                    eng = nc.vector if mode == 'y' else nc.gpsimd
                    eng.tensor_mul(prod, enc_sb[:, bi, :], v_bc[:, b, :])
                    nc.scalar.activation(
                        out=prod,
                        in_=prod,
                        func=mybir.ActivationFunctionType.Copy,
                        bias=0.0,
                        scale=1.0,
                        accum_out=scores[:, b, st:st + 1],
                    )

    # ---- softmax over s, in transposed [BPC, S] layout ------------------
    # scores [128 s_in, (b t)] -> PE transpose -> [(b t), s_in] -> SBUF->SBUF
    # DMA reshape -> s4 [BPC, S].  Then softmax is a single free-axis chain:
    # -max (negated reduce), in-place exp with bias + fused denominator
    # accum, reciprocal, in-place scale, natural-layout store.
    sc2 = scores.rearrange("p b t -> p (b t)")
    scT_ps = tpsum.tile([BPC * NST, 128], f32)
    nc.tensor.transpose(scT_ps[:, :], sc2, ident[:, :])
    scT = small.tile([BPC * NST, 128], f32)
    nc.vector.tensor_copy(scT, scT_ps[:, :])
    s4 = singles.tile([BPC, S], f32)
    nc.sync.dma_start(out=s4, in_=scT)

    nm4 = small.tile([BPC, 1], f32)
    nc.vector.tensor_reduce(
        out=nm4, in_=s4, axis=mybir.AxisListType.X, op=mybir.AluOpType.max,
        negate=True,
    )
    r4 = small.tile([BPC, 1], f32)
    nc.scalar.activation(
        out=s4,
        in_=s4,
        func=mybir.ActivationFunctionType.Exp,
        bias=nm4,
        scale=1.0,
        accum_out=r4,
    )
    inv4 = small.tile([BPC, 1], f32)
    nc.vector.reciprocal(inv4, r4)
    nc.vector.tensor_scalar_mul(s4, s4, inv4)
    nc.sync.dma_start(out=out, in_=s4)


def _declare(nc, S_=None):
    """Declare the per-core DRAM tensors (fp16 inputs, fp32 output)."""
    from concourse import mybir

    S_ = S if S_ is None else S_
    enc_d = nc.dram_tensor("enc", [S_, BPC, K], mybir.dt.float16, kind="ExternalInput")
    hid_d = nc.dram_tensor(
        "hidT", [128, NHC * BPC], mybir.dt.float16, kind="ExternalInput"
    )
    w_d = nc.dram_tensor("w", [H, K], mybir.dt.float16, kind="ExternalInput")
    out_d = nc.dram_tensor(
        "attn_out", [BPC, S_], mybir.dt.float32, kind="ExternalOutput"
    )
    return enc_d, hid_d, w_d, out_d


def _build():
    if "nc" in _CACHE:
        return _CACHE["nc"]
    from contextlib import ExitStack

    import concourse.bacc as bacc
    import concourse.tile as tile

    nc = bacc.Bacc(
        "TRN2", target_bir_lowering=False, debug=False, num_devices=N_CORES
    )
    enc_d, hid_d, w_d, out_d = _declare(nc)

    with tile.TileContext(nc) as tc:
        with ExitStack() as ctx:
            _emit(ctx, tc, enc_d.ap(), hid_d.ap(), w_d.ap(), out_d.ap())
    nc.compile()
    _CACHE["nc"] = nc
    return nc


def _make_core_inputs(hid_bpc, enc_bpc, w16):
    """hid_bpc [BPC, H] fp32/16, enc_bpc [S', BPC, K] -> core in_map (fp16)."""
    hidT = np.ascontiguousarray(
        hid_bpc.T.reshape(NHC, 128, BPC).transpose(1, 0, 2).reshape(128, NHC * BPC)
    ).astype(np.float16)
    enc = np.ascontiguousarray(enc_bpc, dtype=np.float16)
    return {"enc": enc, "hidT": hidT, "w": w16}


def _make_in_maps(hidden, encoder_outputs, W):
    w16 = np.ascontiguousarray(W.astype(np.float16))
    enc16 = encoder_outputs.astype(np.float16)
    in_maps = []
    for i in range(N_CORES):
        b0 = i * BPC
        in_maps.append(
            _make_core_inputs(
                hidden[0, b0:b0 + BPC, :], enc16[:, b0:b0 + BPC, :], w16
            )
        )
    return in_maps


def kernel(hidden, encoder_outputs, W, b):
    from concourse import bass_utils

    nc = _build()
    in_maps = _make_in_maps(
        np.asarray(hidden), np.asarray(encoder_outputs), np.asarray(W)
    )
    res = bass_utils.run_bass_kernel_spmd(
        nc, in_maps, core_ids=list(range(N_CORES))
    )
    out = np.concatenate(
        [res.results[i]["attn_out"] for i in range(N_CORES)], axis=0
    )  # [B, S]
    return out[:, None, :].astype(np.float32)


# revision 15
# speedup vs baseline: 1.6068x; 1.3686x over previous
"""Trainium2 Bass kernel for nn_Attn (Bahdanau-style attention scores).

Reference computation:
    energy[s,b,:] = W @ enc[s,b,:] + bias          [S,B,H]
    scores[b,s]   = hidden[0,b,:] . energy[s,b,:]  [B,S]
    out           = softmax(scores, axis=-1)[:,None,:]

Key rewrite: scores[b,s] = (W^T hidden_b) . enc[s,b,:] + hidden_b . bias.
The second term is constant in s, so it is invariant under softmax and is
dropped entirely.  v_b = W^T hidden_b is a tiny [B, 2H] matvec done on the
tensor engine.

fp16 edition: enc, W, hidden and v are all fp16 (host-side cast), which
halves HBM traffic to ~36 MiB/core (hard floor ~110-115 us at ~330 GB/s).

The S*B*2H dot-product sweep is split BY BATCH across two pipelines so
every engine stays under the DMA streaming time:
  batches 0-1 (natural [s,b,k] layout, s on partitions):
      DVE tensor_mul fp16 (2x mode, ~1.6us/tile) +
      ScalarE activation-Copy accum (~1.7us/tile)      -> scores[s_p, b, t]
  batches 2-3 (host-TRANSPOSED [k,s] layout, k on partitions):
      TensorE matvec: psum[1, 512] += v_kc^T @ encT[kc, s-chunk]
      accumulated over the 16 k-chunks; lands directly in softmax layout.
Engine busy estimate: DVE ~60us, Scalar ~65us, PE ~60-95us, all < DMA.

Sharding: data-parallel over batch B (4 batch rows per core, 8 cores).
Softmax tail unchanged (fp32).
"""

import numpy as np

# Problem sizes (hardcoded per harness contract).
H = 1024          # hidden size
K = 2 * H         # 2H = contraction dim of W
S = 2048          # encoder sequence length
B = 32            # batch
N_CORES = 8
BPC = B // N_CORES  # batch rows per core = 4
NB_E = 2          # batches swept element-wise (DVE+Scalar): b = 0, 1
NB_P = BPC - NB_E  # batches swept on the tensor engine: b = 2, 3

ST = 128          # s-tile (partition dim) for the element-wise sweep
KC = 512          # psum free chunk for the v matmul
NKC = K // KC     # 4
HC = 128          # h chunk (matmul contraction tile)
NHC = H // HC     # 8
NKP = K // 128    # 16 k-chunks of 128 (PE sweep contraction tiles)

_CACHE = {}


def _emit(ctx, tc, enc, enct, hidT, w, out):
    """Emit the per-core program.

    enc : DRAM [S, NB_E, K]  fp16           (batches 0-1, natural layout)
    enct: DRAM [NB_P, NSC, NKP, 128, SCW] fp16 (batches 2-3, transposed)
    hidT: DRAM [128, NHC*BPC] fp16, layout [p][c][b] for h = c*128 + p
    w   : DRAM [H, K] fp16
    out : DRAM [BPC, S] fp32  (softmax probabilities)
    """
    from concourse import mybir
    from concourse.masks import make_identity

    nc = tc.nc
    f32 = mybir.dt.float32
    f16 = mybir.dt.float16

    NST = S // ST          # element-wise s-tiles (16 at full size)
    SCW = min(512, S)      # PE-sweep s-chunk width
    NSC = S // SCW         # PE-sweep s-chunks per batch

    singles = ctx.enter_context(tc.tile_pool(name="singles", bufs=1))
    wpool = ctx.enter_context(tc.tile_pool(name="wpool", bufs=2))
    encpool = ctx.enter_context(tc.tile_pool(name="encp", bufs=6))
    tencpool = ctx.enter_context(tc.tile_pool(name="tencp", bufs=3))
    prodpool = ctx.enter_context(tc.tile_pool(name="prodp", bufs=4))
    vpsum = ctx.enter_context(tc.tile_pool(name="vpsum", bufs=1, space="PSUM"))
    bcpsum = ctx.enter_context(tc.tile_pool(name="bcpsum", bufs=1, space="PSUM"))
    spsum = ctx.enter_context(tc.tile_pool(name="spsum", bufs=2, space="PSUM"))
    tpsum = ctx.enter_context(tc.tile_pool(name="tpsum", bufs=1, space="PSUM"))
    small = ctx.enter_context(tc.tile_pool(name="small", bufs=2))

    # ---- constants (no input deps; scheduled early) ---------------------
    ident = singles.tile([128, 128], f32)
    make_identity(nc, ident)
    ident16 = singles.tile([128, 128], f16)
    make_identity(nc, ident16)
    ones16 = singles.tile([1, 128], f16)
    nc.vector.memset(ones16, 1.0)

    # ---- PE warm-up ------------------------------------------------------
    # TensorE clocks at 1.2 GHz until it has been busy ~3us, then 2.4 GHz.
    # Burn dummy matmuls on a scratch PSUM bank while the W DMAs stream.
    warm_ps = bcpsum.tile([128, KC], f32, name="warm_ps", tag="bc_ps")
    for _ in range(24):
        nc.tensor.matmul(
            warm_ps[:, 0:128], lhsT=ident, rhs=ident, start=True, stop=True
        )

    # ---- load hidden^T (tiny, fp16) -------------------------------------
    hid_sb = singles.tile([128, NHC * BPC], f16)
    nc.scalar.dma_start(out=hid_sb, in_=hidT)

    # ---- v = W^T h, quarter-by-quarter over k ---------------------------
    # W streams as 4 column-quarter tiles [128, NHC, KC] fp16.  Per quarter:
    # matvec into psum -> v16_sb (fp16) -> flatten row -> for b 0-1 a PE
    # ones-matmul broadcast into v_bc; for b 2-3 a strided SBUF DMA into the
    # transposed vT_sb [k_p, kc, b] used as the PE-sweep stationary weights.
    v_bc = singles.tile([128, NB_E, K], f16)
    vT_sb = singles.tile([128, NKP, NB_P], f16)
    v16_sb = singles.tile([BPC, K], f16)
    w_dmas = []
    for q in range(NKC):
        w_sb = wpool.tile([128, NHC, KC], f16, name="w_sb", tag="w_sb")
        weng = nc.scalar if (q % 2 == 0) else nc.sync
        w_dmas.append(
            weng.dma_start(
                out=w_sb,
                in_=w[:, q * KC:(q + 1) * KC].rearrange("(c p) k -> p c k", p=HC),
            )
        )
        v_ps = vpsum.tile([BPC, KC], f32, name="v_ps", tag="v_ps", bufs=1)
        for c in range(NHC):
            nc.tensor.matmul(
                v_ps[:, :],
                lhsT=hid_sb[:, c * BPC:(c + 1) * BPC],
                rhs=w_sb[:, c, :],
                start=(c == 0),
                stop=(c == NHC - 1),
            )
        # downcast to fp16 on the psum->sbuf copy
        nc.scalar.copy(out=v16_sb[:, q * KC:(q + 1) * KC], in_=v_ps[:, :])
        # flatten the 4 v rows of this quarter onto partition 0
        v_row = singles.tile([1, BPC * KC], f16, name="v_row", tag="v_row")
        nc.gpsimd.dma_start(out=v_row, in_=v16_sb[:, q * KC:(q + 1) * KC])
        ncc = KC // 128  # k-chunks of 128 in this quarter
        # v row order is [pe batches..., elementwise batches...] so the
        # transpose lhsT sits at base partition 0 (hw requirement).
        for b in range(NB_E):
            bc_ps = bcpsum.tile([128, KC], f32, name="bc_ps", tag="bc_ps")
            nc.tensor.matmul(
                bc_ps[:, :],
                lhsT=ones16,
                rhs=v_row[0:1, (NB_P + b) * KC:(NB_P + b + 1) * KC],
                start=True,
                stop=True,
            )
            eng = nc.vector if (q * BPC + b) % 2 == 0 else nc.scalar
            if eng is nc.vector:
                eng.tensor_copy(v_bc[:, b, q * KC:(q + 1) * KC], bc_ps[:, :])
            else:
                eng.copy(out=v_bc[:, b, q * KC:(q + 1) * KC], in_=bc_ps[:, :])
        # vT_sb[p, q*ncc + cc, :] = v_{NB_E+b'}[q*KC + cc*128 + p] via PE
        # transpose of the fp16 v rows (2x128 chunks -> psum [128, 2])
        for cc in range(ncc):
            tr_ps = tpsum.tile([128, NB_P], f16, name="tr_ps", tag="tr_ps",
                               bufs=1)
            nc.tensor.transpose(
                tr_ps[:, :],
                v16_sb[0:NB_P,
                       q * KC + cc * 128:q * KC + (cc + 1) * 128],
                ident16[0:NB_P, 0:NB_P],
            )
            eng = nc.vector if cc % 2 == 0 else nc.scalar
            if eng is nc.vector:
                eng.tensor_copy(vT_sb[:, q * ncc + cc, :], tr_ps[:, :])
            else:
                eng.copy(out=vT_sb[:, q * ncc + cc, :], in_=tr_ps[:, :])

    # ---- main sweep ------------------------------------------------------
    # Two interleaved streams share the sync DMA ring roughly in bandwidth
    # ratio (2 natural 1-MiB tiles : 1 transposed 2-MiB tile).
    from concourse.bass import _add_dep_helper

    scores = singles.tile([128, NB_E, NST], f32)
    s4 = singles.tile([NB_E, S], f32)
    sP = [singles.tile([1, S], f32, name=f"sP{i}") for i in range(NB_P)]

    def row_softmax(row, eng_r):
        """Softmax over the free axis of a [1, S] (or [NB_E, S]) tile."""
        p = row.shape[0]
        nm = small.tile([p, 1], f32, name="nm", tag=f"nm{p}", bufs=2)
        eng_r.tensor_reduce(
            out=nm, in_=row, axis=mybir.AxisListType.X,
            op=mybir.AluOpType.max, negate=True,
        )
        r = small.tile([p, 1], f32, name="r", tag=f"r{p}", bufs=2)
        nc.scalar.activation(
            out=row, in_=row, func=mybir.ActivationFunctionType.Exp,
            bias=nm, scale=1.0, accum_out=r,
        )
        inv = small.tile([p, 1], f32, name="inv", tag=f"inv{p}", bufs=2)
        eng_r.reciprocal(inv, r)
        eng_r.tensor_scalar_mul(row, row, inv)

    # generator for the PE-sweep (b, sc) units
    pe_units = [(b, sc) for b in range(NB_P) for sc in range(NSC)]
    pe_i = 0
    held = []

    def emit_pe_unit():
        nonlocal pe_i
        if pe_i >= len(pe_units):
            return
        bp, sc = pe_units[pe_i]
        pe_i += 1
        te = tencpool.tile([128, NKP, SCW], f16, name="te", tag="te")
        d = nc.sync.dma_start(
            out=te, in_=enct[bp, sc].rearrange("kc p s -> p kc s")
        )
        held.append(d)
        chain = spsum.tile([1, SCW], f32, name="chain", tag="chain")
        for kc in range(NKP):
            nc.tensor.matmul(
                chain[:, :],
                lhsT=vT_sb[:, kc, bp:bp + 1],
                rhs=te[:, kc, :],
                start=(kc == 0),
                stop=(kc == NKP - 1),
            )
        eng = nc.vector if (pe_i % 2 == 0) else nc.scalar
        dst = sP[bp][0:1, sc * SCW:(sc + 1) * SCW]
        if eng is nc.vector:
            eng.tensor_copy(dst, chain[:, :])
        else:
            eng.copy(out=dst, in_=chain[:, :])
        if sc == NSC - 1:
            # this batch's scores are complete: emit its softmax chain now
            # so it runs as soon as the last chain copy lands
            row_softmax(sP[bp], nc.vector)
            nc.sync.dma_start(
                out=out[NB_E + bp:NB_E + bp + 1, :], in_=sP[bp]
            )

    for st in range(NST):
        enc_sb = encpool.tile([128, NB_E, K], f16)
        enc_dma = nc.sync.dma_start(
            out=enc_sb,
            in_=enc[st * ST:(st + 1) * ST, :, :],
        )
        held.append(enc_dma)
        for bi in range(NB_E):
            prod = prodpool.tile([128, K], f16, name="prod", tag="prod")
            nc.vector.tensor_mul(prod, enc_sb[:, bi, :], v_bc[:, bi, :])
            nc.scalar.activation(
                out=prod,
                in_=prod,
                func=mybir.ActivationFunctionType.Copy,
                bias=0.0,
                scale=1.0,
                accum_out=scores[:, bi, st:st + 1],
            )
        if st % 2 == 1:
            emit_pe_unit()
    while pe_i < len(pe_units):
        emit_pe_unit()

    # the W phase is DMA-bound (~13us at full rate): hold the first bulk
    # enc DMAs until every W quarter has landed so enc traffic never
    # delays the v chain
    for d in held[:4]:
        _add_dep_helper(
            d.ins, w_dmas[-1].ins, reason="enc stream yields to W prologue"
        )

    # ---- softmax for the element-wise batches ---------------------------
    # scores [128 s_in, (b t)] -> PE transpose -> [(b t), s_in] ->
    # SBUF->SBUF DMA reshape -> s4 [NB_E, S] -> free-axis softmax chain.
    # (PE-swept batches emitted their own chains inside the sweep.)
    sc2 = scores.rearrange("p b t -> p (b t)")
    scT_ps = tpsum.tile([NB_E * NST, 128], f32)
    nc.tensor.transpose(scT_ps[:, :], sc2, ident[:, :])
    scT = small.tile([NB_E * NST, 128], f32)
    nc.vector.tensor_copy(scT, scT_ps[:, :])
    nc.sync.dma_start(out=s4, in_=scT)

    row_softmax(s4, nc.vector)
    nc.sync.dma_start(out=out[0:NB_E, :], in_=s4)


def _declare(nc, S_=None):
    """Declare the per-core DRAM tensors (fp16 inputs, fp32 output)."""
    from concourse import mybir

    S_ = S if S_ is None else S_
    scw = min(512, S_)
    nsc = S_ // scw
    enc_d = nc.dram_tensor(
        "enc", [S_, NB_E, K], mybir.dt.float16, kind="ExternalInput"
    )
    enct_d = nc.dram_tensor(
        "enct", [NB_P, nsc, NKP, 128, scw], mybir.dt.float16,
        kind="ExternalInput",
    )
    hid_d = nc.dram_tensor(
        "hidT", [128, NHC * BPC], mybir.dt.float16, kind="ExternalInput"
    )
    w_d = nc.dram_tensor("w", [H, K], mybir.dt.float16, kind="ExternalInput")
    out_d = nc.dram_tensor(
        "attn_out", [BPC, S_], mybir.dt.float32, kind="ExternalOutput"
    )
    return enc_d, enct_d, hid_d, w_d, out_d


def _build():
    if "nc" in _CACHE:
        return _CACHE["nc"]
    from contextlib import ExitStack

    import concourse.bacc as bacc
    import concourse.tile as tile

    nc = bacc.Bacc(
        "TRN2", target_bir_lowering=False, debug=False, num_devices=N_CORES
    )
    enc_d, enct_d, hid_d, w_d, out_d = _declare(nc)

    with tile.TileContext(nc) as tc:
        with ExitStack() as ctx:
            _emit(
                ctx, tc, enc_d.ap(), enct_d.ap(), hid_d.ap(), w_d.ap(),
                out_d.ap(),
            )
    nc.compile()
    _CACHE["nc"] = nc
    return nc


def _make_core_inputs(hid_bpc, enc_bpc, w16):
    """hid_bpc [BPC, H], enc_bpc [S', BPC, K] fp16 -> core in_map (fp16)."""
    s_ = enc_bpc.shape[0]
    scw = min(512, s_)
    nsc = s_ // scw
    # batch order [pe batches (NB_E..), elementwise batches (0..NB_E-1)] so
    # the v rows for the PE sweep land at base partition 0.
    hid_perm = np.concatenate([hid_bpc[NB_E:], hid_bpc[:NB_E]], axis=0)
    hidT = np.ascontiguousarray(
        hid_perm.T.reshape(NHC, 128, BPC).transpose(1, 0, 2).reshape(128, NHC * BPC)
    ).astype(np.float16)
    enc_n = np.ascontiguousarray(enc_bpc[:, :NB_E, :], dtype=np.float16)
    # enct[b', sc, kc, p, s'] = enc[sc*scw + s', NB_E + b', kc*128 + p]
    enct = np.ascontiguousarray(
        enc_bpc[:, NB_E:, :]
        .reshape(nsc, scw, NB_P, NKP, 128)
        .transpose(2, 0, 3, 4, 1)
        .astype(np.float16)
    )
    return {"enc": enc_n, "enct": enct, "hidT": hidT, "w": w16}


def _make_in_maps(hidden, encoder_outputs, W):
    w16 = np.ascontiguousarray(W.astype(np.float16))
    enc16 = encoder_outputs.astype(np.float16)
    in_maps = []
    for i in range(N_CORES):
        b0 = i * BPC
        in_maps.append(
            _make_core_inputs(
                hidden[0, b0:b0 + BPC, :], enc16[:, b0:b0 + BPC, :], w16
            )
        )
    return in_maps


def kernel(hidden, encoder_outputs, W, b):
    from concourse import bass_utils

    nc = _build()
    in_maps = _make_in_maps(
        np.asarray(hidden), np.asarray(encoder_outputs), np.asarray(W)
    )
    res = bass_utils.run_bass_kernel_spmd(
        nc, in_maps, core_ids=list(range(N_CORES))
    )
    out = np.concatenate(
        [res.results[i]["attn_out"] for i in range(N_CORES)], axis=0
    )  # [B, S]
    return out[:, None, :].astype(np.float32)
